# revision 11
# baseline (speedup 1.0000x reference)
"""DeepEmbedAttention TRN2 kernel — 8-core SPMD.

Sharding: 2 cores per batch (B=4). Each core computes the full k/v chain for
its batch (T=2048) and attention outputs for 4 query chunks of 256 tokens.
Chunk assignment is causally load-balanced: even cores take chunks {0,3,4,7},
odd cores {1,2,5,6}. The single SPMD program processes chunks at canonical
slot positions; everything position-dependent (q columns, causal masks,
chunk-boundary tokens) arrives as per-core input data, so one program serves
all 8 cores. Softmax needs no max-subtraction: scores are tanh-capped to
[-64, 64], so exp() cannot overflow fp32.
"""

import sys

if "/opt/trn_rl_repo" not in sys.path:
    sys.path.insert(0, "/opt/trn_rl_repo")

import numpy as np

B, T, C = 4, 2048, 1024
QD, KV = 256, 32
SCORE_SCALE, CAP_SCALE = 1024.0, 64.0
EPS = 1e-5
N_CORES = 8
P = 128
CHUNK = 256
NSLOT = 4                       # q-chunks per core
TQ = NSLOT * CHUNK              # 1024 canonical query tokens per core
NT = T // P                     # 16 token tiles (full sequence)
NQT = TQ // P                   # 8 canonical query token tiles
CHUNKS = [[0, 3, 4, 7], [1, 2, 5, 6]]   # parity -> global chunk ids
R = [4, 8, 12, 16]              # k-tiles per slot (max over parities)
MINQS = [0, 512, 1024, 1536]    # min chunk start over parities, per slot
NEED_MASK = [(s, kt) for s in range(NSLOT) for kt in range(R[s])
             if P * (kt + 1) > MINQS[s]]
MASK_IDX = {sk: i for i, sk in enumerate(NEED_MASK)}
NMASK = len(NEED_MASK)          # 16
NEG = -1.0e30


def _build_program(nc, tc, a, apply_gb):
    from contextlib import ExitStack

    import concourse.mybir as mybir
    from concourse.masks import make_identity

    f32 = mybir.dt.float32
    Alu = mybir.AluOpType
    Act = mybir.ActivationFunctionType

    xTr = a["xT"].rearrange("(a p) t -> p a t", p=P)        # [128, 8, 2048]
    xqTr = a["xqT"].rearrange("(a p) t -> p a t", p=P)      # [128, 8, 1024]
    xqpr = a["xqprevT"].rearrange("(a p) t -> p a t", p=P)  # [128, 8, 4]
    wqqr = a["wqq"].rearrange("(a p) d -> p a d", p=P)      # [128, 8, 256]
    wkvr = a["wkv"].rearrange("(a p) d -> p a d", p=P)      # [128, 8, 64]
    out_d = a["out"]                                        # [1024, 1024]

    ctx = ExitStack()
    const = ctx.enter_context(tc.tile_pool(name="const", bufs=1))
    pers = ctx.enter_context(tc.tile_pool(name="pers", bufs=1))

    # --- constants ---
    ident = const.tile([P, P], f32, tag="ident")
    make_identity(nc, ident[:])
    ones1 = const.tile([P, 1], f32, tag="ones1")
    nc.gpsimd.memset(ones1[:], 1.0)
    eps_sb = const.tile([P, 1], f32, tag="eps")
    nc.gpsimd.memset(eps_sb[:], EPS)
    wkup = const.tile([KV, QD], f32, tag="wkup")
    nc.sync.dma_start(wkup[:], a["wkup"][:])
    # v_mid lives at base partition 32 inside kvmid; PE needs lhsT/rhs bases
    # to match, so W_vupT is loaded at partitions 32..63 as well.
    wvup64 = const.tile([64, C], f32, tag="wvup")
    nc.sync.dma_start(wvup64[KV:64, :], a["wvup"][:])
    wvup = wvup64[KV:64, :]
    wqq = const.tile([P, 8, QD], f32, tag="wqq")
    nc.sync.dma_start(wqq[:], wqqr[:])
    wkv = const.tile([P, 8, 64], f32, tag="wkv")
    nc.sync.dma_start(wkv[:], wkvr[:])
    xq_rep = const.tile([P, QD], f32, tag="xq_rep")
    nc.sync.dma_start(xq_rep[:], a["xq_rep"][:])
    xk_rep = const.tile([P, QD], f32, tag="xk_rep")
    nc.sync.dma_start(xk_rep[:], a["xk_rep"][:])
    xv_rep = const.tile([P, C], f32, tag="xv_rep")
    nc.sync.dma_start(xv_rep[:], a["xv_rep"][:])
    if apply_gb:
        gb = {}
        for nm, d in [("gq", QD), ("bq", QD), ("gk", QD), ("bk", QD),
                      ("gv", C), ("bv", C)]:
            gb[nm] = const.tile([P, d], f32, tag=nm + "_rep")
            nc.sync.dma_start(gb[nm][:], a[nm + "_rep"][:])

    # --- persistent strips ---
    kvmid = pers.tile([64, T], f32, tag="kvmid")      # [k_mid; v_mid]^T
    qraw = pers.tile([P, NQT, QD], f32, tag="qraw")   # canonical q tiles
    qprev = pers.tile([NSLOT, QD], f32, tag="qprev")  # chunk-boundary q rows
    kk = pers.tile([P, NT, QD], f32, tag="kk")        # k chain, [T, QD] tiles
    vv = pers.tile([P, NT, C], f32, tag="vv")         # v chain, [T, C] tiles
    kT = pers.tile([P, 2, T], f32, tag="kT")          # k^T for attention
    qT = pers.tile([P, 2, TQ], f32, tag="qT")         # q^T for attention

    # ---------------- Phase A: kv_mid + q projections ----------------
    with (tc.tile_pool(name="xin", bufs=3) as xin,
          tc.tile_pool(name="ps_a", bufs=2, space="PSUM") as ps_a):
        for tb in range(T // 512):
            xt = xin.tile([P, 8, 512], f32, tag="xt")
            nc.sync.dma_start(xt[:], xTr[:, :, tb * 512:(tb + 1) * 512])
            kvps = ps_a.tile([64, 512], f32, tag="kvps")
            for cc in range(8):
                nc.tensor.matmul(kvps[:], wkv[:, cc, :], xt[:, cc, :],
                                 start=(cc == 0), stop=(cc == 7))
            nc.scalar.copy(kvmid[:, tb * 512:(tb + 1) * 512], kvps[:])

        for tt in range(NQT):
            xqt = xin.tile([P, 8, P], f32, tag="xqt")
            nc.sync.dma_start(xqt[:], xqTr[:, :, tt * P:(tt + 1) * P])
            qps = ps_a.tile([P, QD], f32, tag="qps")
            for cc in range(8):
                nc.tensor.matmul(qps[:], xqt[:, cc, :], wqq[:, cc, :],
                                 start=(cc == 0), stop=(cc == 7))
            nc.scalar.copy(qraw[:, tt, :], qps[:])

        xqp = xin.tile([P, 8, NSLOT], f32, tag="xqp")
        nc.sync.dma_start(xqp[:], xqpr[:])
        qpps = ps_a.tile([NSLOT, QD], f32, tag="qpps")
        for cc in range(8):
            nc.tensor.matmul(qpps[:], xqp[:, cc, :], wqq[:, cc, :],
                             start=(cc == 0), stop=(cc == 7))
        nc.scalar.copy(qprev[:], qpps[:])

    # ---------------- Phase B: k/v up-proj + embeddings ----------------
    with (tc.tile_pool(name="emb", bufs=3) as embp,
          tc.tile_pool(name="ps_b", bufs=2, space="PSUM") as ps_b):
        for tt in range(NT):
            kps = ps_b.tile([P, QD], f32, tag="kps")
            nc.tensor.matmul(kps[:], kvmid[0:KV, tt * P:(tt + 1) * P],
                             wkup[:], start=True, stop=True)
            kemb = embp.tile([P, QD], f32, tag="kemb")
            nc.sync.dma_start(kemb[:], a["kemb"][tt * P:(tt + 1) * P, :])
            nc.vector.tensor_tensor(out=kk[:, tt, :], in0=kps[:], in1=kemb[:],
                                    op=Alu.mult)

            vps = ps_b.tile([P, C], f32, tag="vps")
            nc.tensor.matmul(vps[:, 0:512], kvmid[KV:64, tt * P:(tt + 1) * P],
                             wvup[:, 0:512], start=True, stop=True)
            nc.tensor.matmul(vps[:, 512:C], kvmid[KV:64, tt * P:(tt + 1) * P],
                             wvup[:, 512:C], start=True, stop=True)
            vtmp = embp.tile([P, C], f32, tag="vtanh")
            nc.scalar.activation(vtmp[:], vps[:], Act.Tanh)
            vemb = embp.tile([P, C], f32, tag="vemb")
            nc.sync.dma_start(vemb[:], a["vemb"][tt * P:(tt + 1) * P, :])
            nc.vector.tensor_tensor(out=vv[:, tt, :], in0=vtmp[:], in1=vemb[:],
                                    op=Alu.mult)

    # ---------------- Phase C: shift + blend + layernorm ----------------
    # Tiles are processed in DESCENDING order: tile tt's shift reads the
    # pre-blend row 127 of tile tt-1, which is only untouched if tt-1 has
    # not been blended in place yet. Tile's conservative dependency
    # tracking preserves this trace order on the overlapping slices.
    def blend_ln(buf, tt, d, coef, sh_tag, bnd_src, g=None, b=None):
        with_stats_split = d > 512
        sh = shp.tile([P, d], f32, tag=sh_tag)
        nc.sync.dma_start(sh[1:P, :], buf[0:P - 1, tt, :])
        if bnd_src is None:
            nc.vector.memset(sh[0:1, :], 0.0)
        else:
            nc.sync.dma_start(sh[0:1, :], bnd_src)
        nc.vector.tensor_tensor(out=sh[:], in0=sh[:], in1=buf[:, tt, :],
                                op=Alu.subtract)
        nc.vector.tensor_tensor(out=sh[:], in0=sh[:], in1=coef[:],
                                op=Alu.mult)
        nc.vector.tensor_tensor(out=buf[:, tt, :], in0=buf[:, tt, :],
                                in1=sh[:], op=Alu.add)
        if with_stats_split:
            st = shp.tile([P, 2, 6], f32, tag=sh_tag + "st")
            bufr = buf[:, tt, :].rearrange("p (a d) -> p a d", d=512)
            nc.vector.bn_stats(out=st[:, 0, :], in_=bufr[:, 0, :])
            nc.vector.bn_stats(out=st[:, 1, :], in_=bufr[:, 1, :])
        else:
            st = shp.tile([P, 6], f32, tag=sh_tag + "st")
            nc.vector.bn_stats(out=st[:], in_=buf[:, tt, :])
        mv = shp.tile([P, 2], f32, tag=sh_tag + "mv")
        nc.vector.bn_aggr(out=mv[:], in_=st[:])
        nc.scalar.activation(mv[:, 1:2], mv[:, 1:2], Act.Sqrt, bias=eps_sb[:])
        nc.vector.reciprocal(mv[:, 1:2], mv[:, 1:2])
        nc.vector.tensor_scalar(out=buf[:, tt, :], in0=buf[:, tt, :],
                                scalar1=mv[:, 0:1], scalar2=mv[:, 1:2],
                                op0=Alu.subtract, op1=Alu.mult)
        if g is not None:
            nc.vector.tensor_tensor(out=buf[:, tt, :], in0=buf[:, tt, :],
                                    in1=g[:], op=Alu.mult)
            nc.vector.tensor_tensor(out=buf[:, tt, :], in0=buf[:, tt, :],
                                    in1=b[:], op=Alu.add)

    gq = gb["gq"] if apply_gb else None
    bq = gb["bq"] if apply_gb else None
    gk = gb["gk"] if apply_gb else None
    bk = gb["bk"] if apply_gb else None
    gv = gb["gv"] if apply_gb else None
    bv = gb["bv"] if apply_gb else None

    with (tc.tile_pool(name="shp", bufs=3) as shp,
          tc.tile_pool(name="ps_t", bufs=2, space="PSUM") as ps_t):
        for tt in range(NT - 1, -1, -1):
            blend_ln(kk, tt, QD, xk_rep, "ksh",
                     None if tt == 0 else kk[P - 1:P, tt - 1, :], gk, bk)
            blend_ln(vv, tt, C, xv_rep, "vsh",
                     None if tt == 0 else vv[P - 1:P, tt - 1, :], gv, bv)
        for tt in range(NQT - 1, -1, -1):
            src = (qprev[tt // 2:tt // 2 + 1, :] if tt % 2 == 0
                   else qraw[P - 1:P, tt - 1, :])
            blend_ln(qraw, tt, QD, xq_rep, "qsh", src, gq, bq)

        # transposes into attention layouts
        for tt in range(NT):
            for qc in range(2):
                tps = ps_t.tile([P, P], f32, tag="tps")
                nc.tensor.transpose(tps[:], kk[:, tt, qc * P:(qc + 1) * P],
                                    ident[:])
                nc.vector.tensor_copy(out=kT[:, qc, tt * P:(tt + 1) * P],
                                      in_=tps[:])
        for tt in range(NQT):
            for qc in range(2):
                tps = ps_t.tile([P, P], f32, tag="tps")
                nc.tensor.transpose(tps[:], qraw[:, tt, qc * P:(qc + 1) * P],
                                    ident[:])
                nc.vector.tensor_copy(out=qT[:, qc, tt * P:(tt + 1) * P],
                                      in_=tps[:])

    # ---------------- Phase D: attention ----------------
    with (tc.tile_pool(name="att", bufs=4) as attp,
          tc.tile_pool(name="mask", bufs=3) as maskp,
          tc.tile_pool(name="outs", bufs=3) as outsp,
          tc.tile_pool(name="ps_sc", bufs=2, space="PSUM") as ps_sc,
          tc.tile_pool(name="ps_out", bufs=1, space="PSUM") as ps_out,
          tc.tile_pool(name="ps_sum", bufs=1, space="PSUM") as ps_sum):
        for s in range(NSLOT):
            sums = [ps_sum.tile([P, 1], f32, tag=f"sums{i}",
                                name=f"sums_{s}_{i}") for i in range(2)]
            ops = [ps_out.tile([P, 512], f32, tag=f"o{i}{ch}",
                               name=f"ops_{s}_{i}{ch}")
                   for i in range(2) for ch in range(2)]
            for kt in range(R[s]):
                sps = ps_sc.tile([P, CHUNK], f32, tag="sps")
                for qc in range(2):
                    nc.tensor.matmul(sps[:], kT[:, qc, kt * P:(kt + 1) * P],
                                     qT[:, qc, s * CHUNK:(s + 1) * CHUNK],
                                     start=(qc == 0), stop=(qc == 1))
                et = attp.tile([P, CHUNK], f32, tag="et")
                nc.scalar.activation(et[:], sps[:], Act.Tanh,
                                     scale=1.0 / SCORE_SCALE)
                if (s, kt) in MASK_IDX:
                    mt = maskp.tile([P, CHUNK], f32, tag="mt")
                    nc.sync.dma_start(mt[:], a["mask"][MASK_IDX[(s, kt)]])
                    nc.vector.tensor_tensor(out=et[:], in0=et[:], in1=mt[:],
                                            op=Alu.add)
                ee = attp.tile([P, CHUNK], f32, tag="ee")
                nc.scalar.activation(ee[:], et[:], Act.Exp, scale=CAP_SCALE)
                first, last = kt == 0, kt == R[s] - 1
                for i in range(2):
                    nc.tensor.matmul(sums[i][:],
                                     ee[:, i * P:(i + 1) * P], ones1[:],
                                     start=first, stop=last)
                    for ch in range(2):
                        nc.tensor.matmul(ops[2 * i + ch][:],
                                         ee[:, i * P:(i + 1) * P],
                                         vv[:, kt, ch * 512:(ch + 1) * 512],
                                         start=first, stop=last)
            recip = attp.tile([P, 2], f32, tag="recip")
            for i in range(2):
                nc.vector.reciprocal(recip[:, i:i + 1], sums[i][:])
            for i in range(2):
                for ch in range(2):
                    ot = outsp.tile([P, 512], f32, tag="ot")
                    nc.vector.tensor_scalar_mul(out=ot[:],
                                                in0=ops[2 * i + ch][:],
                                                scalar1=recip[:, i:i + 1])
                    nc.sync.dma_start(
                        out_d[s * CHUNK + i * P:s * CHUNK + (i + 1) * P,
                              ch * 512:(ch + 1) * 512], ot[:])

    ctx.close()


_NC_CACHE = {}


def _input_specs(apply_gb):
    import concourse.mybir as mybir
    f32 = mybir.dt.float32
    specs = [
        ("xT", [C, T], f32), ("xqT", [C, TQ], f32),
        ("xqprevT", [C, NSLOT], f32),
        ("kemb", [T, QD], f32), ("vemb", [T, C], f32),
        ("wqq", [C, QD], f32), ("wkv", [C, 64], f32),
        ("wkup", [KV, QD], f32), ("wvup", [KV, C], f32),
        ("xq_rep", [P, QD], f32), ("xk_rep", [P, QD], f32),
        ("xv_rep", [P, C], f32),
        ("mask", [NMASK, P, CHUNK], f32),
    ]
    if apply_gb:
        specs += [("gq_rep", [P, QD], f32), ("bq_rep", [P, QD], f32),
                  ("gk_rep", [P, QD], f32), ("bk_rep", [P, QD], f32),
                  ("gv_rep", [P, C], f32), ("bv_rep", [P, C], f32)]
    return specs


def get_nc(apply_gb):
    key = bool(apply_gb)
    if key in _NC_CACHE:
        return _NC_CACHE[key]
    import concourse.mybir as mybir
    import concourse.tile as tile
    from concourse import bacc

    nc = bacc.Bacc("TRN2", target_bir_lowering=False, debug=False,
                   num_devices=N_CORES)
    a = {}
    for name, shape, dt in _input_specs(apply_gb):
        a[name] = nc.dram_tensor(name, shape, dt, kind="ExternalInput").ap()
    a["out"] = nc.dram_tensor("out", [TQ, C], mybir.dt.float32,
                              kind="ExternalOutput").ap()
    with tile.TileContext(nc) as tc:
        _build_program(nc, tc, a, apply_gb)
    nc.compile()
    _NC_CACHE[key] = nc
    return nc


def _parity_mask(parity):
    m = np.zeros((NMASK, P, CHUNK), np.float32)
    for (s, kt), mi in MASK_IDX.items():
        qs = CHUNKS[parity][s] * CHUNK
        kg = np.arange(P, dtype=np.int64)[:, None] + P * kt
        qg = np.arange(CHUNK, dtype=np.int64)[None, :] + qs
        m[mi] = np.where(qg >= kg, 0.0, NEG).astype(np.float32)
    return m


def make_in_maps(inputs):
    x = np.asarray(inputs["x"], np.float32)
    idx = np.asarray(inputs["idx"]).astype(np.int64)
    k_tab = np.asarray(inputs["k_emb_tab"], np.float32)
    v_tab = np.asarray(inputs["v_emb_tab"], np.float32)
    W_qq = np.asarray(inputs["W_qq"], np.float32)
    W_k = np.asarray(inputs["W_k"], np.float32)
    W_kup = np.asarray(inputs["W_kup"], np.float32)
    W_v = np.asarray(inputs["W_v"], np.float32)
    W_vup = np.asarray(inputs["W_vup"], np.float32)
    x_q = np.asarray(inputs["x_q"], np.float32).reshape(QD)
    x_k = np.asarray(inputs["x_k"], np.float32).reshape(QD)
    x_v = np.asarray(inputs["x_v"], np.float32).reshape(C)
    g_q = np.asarray(inputs["g_q"], np.float32).reshape(QD)
    b_q = np.asarray(inputs["b_q"], np.float32).reshape(QD)
    g_k = np.asarray(inputs["g_k"], np.float32).reshape(QD)
    b_k = np.asarray(inputs["b_k"], np.float32).reshape(QD)
    g_v = np.asarray(inputs["g_v"], np.float32).reshape(C)
    b_v = np.asarray(inputs["b_v"], np.float32).reshape(C)

    apply_gb = not (np.all(g_q == 1) and np.all(b_q == 0)
                    and np.all(g_k == 1) and np.all(b_k == 0)
                    and np.all(g_v == 1) and np.all(b_v == 0))

    k_emb = k_tab[idx]          # [B, T, QD]
    v_emb = v_tab[idx]          # [B, T, C]

    shared = {
        "wqq": np.ascontiguousarray(W_qq.T),
        "wkv": np.ascontiguousarray(np.concatenate([W_k, W_v], 0).T),
        "wkup": np.ascontiguousarray(W_kup.T),
        "wvup": np.ascontiguousarray(W_vup.T),
        "xq_rep": np.ascontiguousarray(np.broadcast_to(x_q, (P, QD))),
        "xk_rep": np.ascontiguousarray(np.broadcast_to(x_k, (P, QD))),
        "xv_rep": np.ascontiguousarray(np.broadcast_to(x_v, (P, C))),
    }
    if apply_gb:
        for nm, v in [("gq", g_q), ("bq", b_q), ("gk", g_k), ("bk", b_k)]:
            shared[nm + "_rep"] = np.ascontiguousarray(
                np.broadcast_to(v, (P, QD)))
        for nm, v in [("gv", g_v), ("bv", b_v)]:
            shared[nm + "_rep"] = np.ascontiguousarray(
                np.broadcast_to(v, (P, C)))

    pmask = [_parity_mask(0), _parity_mask(1)]
    in_maps = []
    for c in range(N_CORES):
        b, parity = c // 2, c % 2
        chunks = CHUNKS[parity]
        xT = np.ascontiguousarray(x[b].T)
        cols = np.concatenate([np.arange(ch * CHUNK, (ch + 1) * CHUNK)
                               for ch in chunks])
        xqT = np.ascontiguousarray(x[b][cols].T)
        xqprev = np.zeros((NSLOT, C), np.float32)
        for j, ch in enumerate(chunks):
            if ch > 0:
                xqprev[j] = x[b, ch * CHUNK - 1]
        m = dict(shared)
        m.update(
            xT=xT, xqT=xqT,
            xqprevT=np.ascontiguousarray(xqprev.T),
            kemb=np.ascontiguousarray(k_emb[b]),
            vemb=np.ascontiguousarray(v_emb[b]),
            mask=pmask[parity],
        )
        in_maps.append(m)
    return in_maps, apply_gb


def assemble_output(results):
    out = np.empty((B, T, C), np.float32)
    for c in range(N_CORES):
        oc = results[c]["out"]
        for j, ch in enumerate(CHUNKS[c % 2]):
            out[c // 2, ch * CHUNK:(ch + 1) * CHUNK] = \
                oc[j * CHUNK:(j + 1) * CHUNK]
    return out


def kernel(**inputs):
    from concourse.bass_utils import run_bass_kernel_spmd
    in_maps, apply_gb = make_in_maps(inputs)
    nc = get_nc(apply_gb)
    res = run_bass_kernel_spmd(nc, in_maps, core_ids=list(range(N_CORES)))
    return assemble_output(res.results)


# revision 32
# speedup vs baseline: 32.0105x; 32.0105x over previous
"""DeepEmbedAttention TRN2 kernel — 8-core SPMD.

Sharding: 2 cores per batch (B=4). Each core computes the full k/v chain for
its batch (T=2048) and attention outputs for 4 query chunks of 256 tokens.
Chunk assignment is causally load-balanced: even cores take chunks {0,3,4,7},
odd cores {1,2,5,6}. The single SPMD program processes chunks at canonical
slot positions; everything position-dependent (q columns, causal masks,
chunk-boundary tokens) arrives as per-core input data, so one program serves
all 8 cores. Softmax needs no max-subtraction: scores are tanh-capped to
[-64, 64], so exp() cannot overflow fp32.

Engine plan: token-shift is done with PE matmuls against constant
superdiagonal/boundary selector matrices (DMA-free). DMAs are batched large
and spread over the three issuing queues (sync = input streams, scalar =
output stores, gpsimd = constants); SBUF-only elementwise work is offloaded
to the otherwise-idle GpSimd engine.
"""

import sys

if "/opt/trn_rl_repo" not in sys.path:
    sys.path.insert(0, "/opt/trn_rl_repo")

import numpy as np

B, T, C = 4, 2048, 1024
QD, KV = 256, 32
SCORE_SCALE, CAP_SCALE = 1024.0, 64.0
EPS = 1e-5
N_CORES = 8
P = 128
CHUNK = 256
NSLOT = 4                       # q-chunks per core
TQ = NSLOT * CHUNK              # 1024 canonical query tokens per core
NT = T // P                     # 16 token tiles (full sequence)
NQT = TQ // P                   # 8 canonical query token tiles
CHUNKS = [[0, 3, 4, 7], [1, 2, 5, 6]]   # parity -> global chunk ids
R = [4, 8, 12, 16]              # k-tiles per slot (max over parities)
MINQS = [0, 512, 1024, 1536]    # min chunk start over parities, per slot
NEED_MASK = [(s, kt) for s in range(NSLOT) for kt in range(R[s])
             if P * (kt + 1) > MINQS[s]]
MASK_IDX = {sk: i for i, sk in enumerate(NEED_MASK)}
NMASK = len(NEED_MASK)          # 16
NEG = -1.0e30


def _build_program(nc, tc, a, apply_gb, bf16, nrep=1):
    from contextlib import ExitStack

    import concourse.mybir as mybir
    from concourse.masks import make_identity

    f32 = mybir.dt.float32
    DT = mybir.dt.bfloat16 if bf16 else f32
    NMAX = 512                      # psum-bank limit caps matmul free size
    Alu = mybir.AluOpType
    Act = mybir.ActivationFunctionType

    xTr = a["xT"].rearrange("(a p) t -> p a t", p=P)        # [128, 8, 2048]
    xqTr = a["xqT"].rearrange("(a p) t -> p a t", p=P)      # [128, 8, 1024]
    xqpr = a["xqprevT"].rearrange("(a p) t -> p a t", p=P)  # [128, 8, 4]
    wqqr = a["wqq"].rearrange("(a p) d -> p a d", p=P)      # [128, 8, 256]
    wkvr = a["wkv"].rearrange("(a p) d -> p a d", p=P)      # [128, 8, 64]
    kembr = a["kemb"].rearrange("(g p) d -> p g d", p=P)    # [128, 16, 256]
    vembr1 = a["vemb1"].rearrange("(g p) d -> p g d", p=P)  # [128, 16, 1024]
    vembr2 = a["vemb2"].rearrange("(g p) d -> p g d", p=P)
    maskr = a["mask"].rearrange("m p q -> p m q")           # [128, 16, 256]
    out_d = a["out"]                                        # [1024, 1024]

    ctx = ExitStack()
    const = ctx.enter_context(tc.tile_pool(name="const", bufs=1))
    pers = ctx.enter_context(tc.tile_pool(name="pers", bufs=1))

    # --- constants (gpsimd queue for the DMAs) ---
    ident = const.tile([P, P], DT, tag="ident")
    make_identity(nc, ident[:])
    # ssup[p, m] = 1 iff m == p+1 : shift-down-one (sh[m] = v[m-1])
    ssup = const.tile([P, P], DT, tag="ssup")
    nc.gpsimd.memset(ssup[:], 0.0)
    nc.gpsimd.affine_select(out=ssup[:], in_=ssup[:],
                            compare_op=Alu.not_equal, fill=1.0,
                            base=1, pattern=[[-1, P]], channel_multiplier=1)
    # bnd[p, m] = 1 iff (p==127, m==0) : carry prev tile's last row into row 0
    bnd = const.tile([P, P], DT, tag="bnd")
    nc.gpsimd.memset(bnd[:], 0.0)
    nc.gpsimd.affine_select(out=bnd[:], in_=bnd[:],
                            compare_op=Alu.not_equal, fill=1.0,
                            base=-(P - 1), pattern=[[-P, P]],
                            channel_multiplier=1)
    # qsel[s][p, m] = 1 iff (p==s, m==0) : qprev row s into row 0
    qsel = []
    for s in range(NSLOT):
        qs_t = const.tile([NSLOT, P], DT, tag=f"qsel{s}", name=f"qsel{s}")
        nc.gpsimd.memset(qs_t[:], 0.0)
        nc.gpsimd.affine_select(out=qs_t[:], in_=qs_t[:],
                                compare_op=Alu.not_equal, fill=1.0,
                                base=-s, pattern=[[-NSLOT, P]],
                                channel_multiplier=1)
        qsel.append(qs_t)
    ones1 = const.tile([P, 1], DT, tag="ones1")
    nc.gpsimd.memset(ones1[:], 1.0)
    eps_sb = const.tile([P, 1], f32, tag="eps")
    nc.gpsimd.memset(eps_sb[:], EPS)

    wkup = const.tile([KV, QD], DT, tag="wkup")
    nc.gpsimd.dma_start(wkup[:], a["wkup"][:])
    # v_mid lives at base partition 32 inside kvmid; PE needs lhsT/rhs bases
    # to match, so W_vupT is loaded at partitions 32..63 as well.
    wvup64 = const.tile([64, C], DT, tag="wvup")
    nc.gpsimd.dma_start(wvup64[KV:64, :], a["wvup"][:])
    wvup = wvup64[KV:64, :]
    wqq = const.tile([P, 8, QD], DT, tag="wqq")
    nc.gpsimd.dma_start(wqq[:], wqqr[:])
    wkv = const.tile([P, 8, 64], DT, tag="wkv")
    nc.gpsimd.dma_start(wkv[:], wkvr[:])
    xq_rep = const.tile([P, QD], DT, tag="xq_rep")
    nc.gpsimd.dma_start(xq_rep[:], a["xq_rep"][:])
    xk_rep = const.tile([P, QD], DT, tag="xk_rep")
    nc.gpsimd.dma_start(xk_rep[:], a["xk_rep"][:])
    xv_rep = const.tile([P, C], DT, tag="xv_rep")
    nc.gpsimd.dma_start(xv_rep[:], a["xv_rep"][:])
    maskall = const.tile([P, NMASK, CHUNK], DT, tag="maskall")
    nc.gpsimd.dma_start(maskall[:], maskr[:])
    gb = {}
    if apply_gb:
        for nm, d in [("gq", QD), ("bq", QD), ("gk", QD), ("bk", QD),
                      ("gv", C), ("bv", C)]:
            gb[nm] = const.tile([P, d], DT, tag=nm + "_rep", name=nm + "_rep")
            nc.gpsimd.dma_start(gb[nm][:], a[nm + "_rep"][:])

    loop = tc.For_i(0, nrep, 1) if nrep > 1 else None
    if loop is not None:
        loop.__enter__()

    # --- persistent strips ---
    kvmid = pers.tile([64, T], DT, tag="kvmid")       # [k_mid; v_mid]^T
    qraw = pers.tile([P, NQT, QD], DT, tag="qraw")    # canonical q tiles
    qprev = pers.tile([NSLOT, QD], DT, tag="qprev")   # chunk-boundary q rows
    kk = pers.tile([P, NT, QD], DT, tag="kk")         # k chain, [T, QD] tiles
    vv = pers.tile([P, NT, C], DT, tag="vv")          # v chain, [T, C] tiles
    kT = pers.tile([P, 2, T], DT, tag="kT")           # k^T for attention
    qT = pers.tile([P, 2, TQ], DT, tag="qT")          # q^T for attention
    statv = pers.tile([P, 2, NT], f32, tag="statv")   # LN (mean, var) strips
    statk = pers.tile([P, 2, NT], f32, tag="statk")
    statq = pers.tile([P, 2, NQT], f32, tag="statq")

    gq, bq = (gb.get("gq"), gb.get("bq"))
    gk, bk = (gb.get("gk"), gb.get("bk"))
    gv, bv = (gb.get("gv"), gb.get("bv"))

    # ---------------- Phase A: kv_mid + q projections ----------------
    with (tc.tile_pool(name="xin", bufs=(4 if bf16 else 2)) as xin,
          tc.tile_pool(name="ps_a", bufs=2, space="PSUM") as ps_a):
        for tb in range(T // 512):
            xt = xin.tile([P, 8, 512], DT, tag="xt")
            nc.sync.dma_start(xt[:], xTr[:, :, tb * 512:(tb + 1) * 512])
            kvps = ps_a.tile([64, 512], f32, tag="kvps")
            for cc in range(8):
                nc.tensor.matmul(kvps[:], wkv[:, cc, :], xt[:, cc, :],
                                 start=(cc == 0), stop=(cc == 7))
            nc.scalar.copy(kvmid[:, tb * 512:(tb + 1) * 512], kvps[:])

        for th in range(2):     # canonical q in two 512-token halves
            xqt = xin.tile([P, 8, 512], DT, tag="xt", name="xqt")
            nc.sync.dma_start(xqt[:], xqTr[:, :, th * 512:(th + 1) * 512])
            for j in range(4):
                tt = th * 4 + j
                qps = ps_a.tile([P, QD], f32, tag="qps")
                for cc in range(8):
                    nc.tensor.matmul(qps[:], xqt[:, cc, j * P:(j + 1) * P],
                                     wqq[:, cc, :],
                                     start=(cc == 0), stop=(cc == 7))
                nc.scalar.copy(qraw[:, tt, :], qps[:])

        xqp = xin.tile([P, 8, NSLOT], DT, tag="xqp")
        nc.sync.dma_start(xqp[:], xqpr[:])
        qpps = ps_a.tile([NSLOT, QD], f32, tag="qpps")
        for cc in range(8):
            nc.tensor.matmul(qpps[:], xqp[:, cc, :], wqq[:, cc, :],
                             start=(cc == 0), stop=(cc == 7))
        nc.scalar.copy(qprev[:], qpps[:])

    # ---------------- Phase B: k up-proj + embeddings ----------------
    with (tc.tile_pool(name="emb", bufs=2) as embp,
          tc.tile_pool(name="ps_b", bufs=2, space="PSUM") as ps_b):
        for g in range(NT // 4):
            kemb = embp.tile([P, 4, QD], DT, tag="kemb")
            nc.scalar.dma_start(kemb[:], kembr[:, g * 4:(g + 1) * 4, :])
            for j in range(4):
                tt = g * 4 + j
                kps = ps_b.tile([P, QD], f32, tag="kps")
                nc.tensor.matmul(kps[:], kvmid[0:KV, tt * P:(tt + 1) * P],
                                 wkup[:], start=True, stop=True)
                nc.vector.tensor_tensor(out=kk[:, tt, :], in0=kps[:],
                                        in1=kemb[:, j, :], op=Alu.mult)

    # ------- v chain: up-proj + pre-scaled embeddings + shift + LN -------
    # vemb1 = v_emb*(1-x_v), vemb2 = v_emb*x_v (host-prescaled), so
    # blended = tanh(proj)*vemb1 + S(tanh(proj)*vemb2): the blend collapses
    # into two SBUF multiplies plus the shift matmul, no sub/mul/add chain.
    # Ascending order; v2 of the previous tile feeds the boundary matmul.
    with (tc.tile_pool(name="vemb", bufs=2) as vembp,
          tc.tile_pool(name="vwork", bufs=3) as vwork,
          tc.tile_pool(name="ps_v", bufs=2, space="PSUM") as ps_v):
        v2_prev = None
        for tt in range(NT):
            g, j = tt // 4, tt % 4
            if j == 0:
                vemb1 = vembp.tile([P, 4, C], DT, tag="vemb1",
                                   name=f"vemb1_{g}")
                nc.scalar.dma_start(vemb1[:],
                                    vembr1[:, g * 4:(g + 1) * 4, :])
                vemb2 = vembp.tile([P, 4, C], DT, tag="vemb2",
                                   name=f"vemb2_{g}")
                nc.scalar.dma_start(vemb2[:],
                                    vembr2[:, g * 4:(g + 1) * 4, :])
            vps = ps_v.tile([P, C], f32, tag="vps")
            for ch in range(2):
                nc.tensor.matmul(vps[:, ch * 512:(ch + 1) * 512],
                                 kvmid[KV:64, tt * P:(tt + 1) * P],
                                 wvup[:, ch * 512:(ch + 1) * 512],
                                 start=True, stop=True)
            vt = vwork.tile([P, C], DT, tag="vt", name=f"vt{tt}")
            nc.scalar.activation(vt[:], vps[:], Act.Tanh)
            v2 = vwork.tile([P, C], DT, tag="v2", name=f"v2_{tt}")
            nc.vector.tensor_tensor(out=vv[:, tt, :], in0=vt[:],
                                    in1=vemb1[:, j, :], op=Alu.mult)
            nc.vector.tensor_tensor(out=v2[:], in0=vt[:],
                                    in1=vemb2[:, j, :], op=Alu.mult)
            shps = ps_v.tile([P, C], f32, tag="shps", name=f"vsh{tt}")
            for ch in range(0, C, NMAX):
                ce = ch + NMAX
                nc.tensor.matmul(shps[:, ch:ce], ssup[:], v2[:, ch:ce],
                                 start=True, stop=v2_prev is None)
                if v2_prev is not None:
                    nc.tensor.matmul(shps[:, ch:ce], bnd[:],
                                     v2_prev[:, ch:ce],
                                     start=False, stop=True)
            nc.vector.tensor_tensor(out=vv[:, tt, :], in0=shps[:],
                                    in1=vv[:, tt, :], op=Alu.add)
            v2_prev = v2
            # LN stats on ACT via accum_out (frees DVE's bn_stats). Sqrt is
            # deferred and batched: it lives in a different ACT function
            # table than Tanh/Exp/Square/Copy, and per-tile use would force
            # a ~1.3us table reload per switch.
            scr = vwork.tile([P, C], DT, tag="vscr", name=f"vscr{tt}")
            ssq = vwork.tile([P, 1], f32, tag="vssq", name=f"vssq{tt}")
            nc.scalar.activation(scr[:], vv[:, tt, :], Act.Square,
                                 accum_out=ssq[:])
            ssm = vwork.tile([P, 1], f32, tag="vssm", name=f"vssm{tt}")
            nc.scalar.activation(scr[:], vv[:, tt, :], Act.Copy,
                                 accum_out=ssm[:])
            nc.vector.tensor_scalar_mul(out=statv[:, 0, tt:tt + 1],
                                        in0=ssm[:], scalar1=1.0 / C)
            msq = vwork.tile([P, 1], f32, tag="vmsq", name=f"vmsq{tt}")
            nc.vector.tensor_tensor(out=msq[:], in0=statv[:, 0, tt:tt + 1],
                                    in1=statv[:, 0, tt:tt + 1], op=Alu.mult)
            nc.vector.tensor_scalar_mul(out=statv[:, 1, tt:tt + 1],
                                        in0=ssq[:], scalar1=1.0 / C)
            nc.vector.tensor_tensor(out=statv[:, 1, tt:tt + 1],
                                    in0=statv[:, 1, tt:tt + 1],
                                    in1=msq[:], op=Alu.subtract)

        nc.scalar.activation(statv[:, 1, :], statv[:, 1, :], Act.Sqrt,
                             bias=eps_sb[:])
        nc.vector.reciprocal(statv[:, 1, :], statv[:, 1, :])
        for tt in range(NT):
            nc.vector.tensor_scalar(out=vv[:, tt, :], in0=vv[:, tt, :],
                                    scalar1=statv[:, 0, tt:tt + 1],
                                    scalar2=statv[:, 1, tt:tt + 1],
                                    op0=Alu.subtract, op1=Alu.mult)
            if gv is not None:
                nc.gpsimd.tensor_tensor(out=vv[:, tt, :], in0=vv[:, tt, :],
                                        in1=gv[:], op=Alu.mult)
                nc.gpsimd.tensor_tensor(out=vv[:, tt, :], in0=vv[:, tt, :],
                                        in1=bv[:], op=Alu.add)

    # ---------------- Phase C: shift + blend + layernorm ----------------
    # Token shift via PE: sh = ssup.T @ tile (+ bnd.T @ prev_tile). Tiles are
    # processed in DESCENDING order so the in-place blend never clobbers a
    # row that a later (earlier-indexed) tile still needs pre-blend.
    def blend_stats(buf, tt, d, coef, nm, prev_rhs, statbuf):
        ps_tag = "shps%d" % d        # k and q share the 256-wide psum tag
        shps = ps_c.tile([P, d], f32, tag=ps_tag, name=nm + f"ps{tt}")
        for ch in range(0, d, NMAX):
            ce = min(ch + NMAX, d)
            nc.tensor.matmul(shps[:, ch:ce], ssup[:], buf[:, tt, ch:ce],
                             start=True, stop=prev_rhs is None)
            if prev_rhs is not None:
                nc.tensor.matmul(shps[:, ch:ce], prev_rhs[0],
                                 prev_rhs[1][:, ch:ce],
                                 start=False, stop=True)
        # gpsimd (Pool) takes the SBUF-only blend ops for these 256-wide
        # tiles; the psum-reading subtract stays on DVE.
        tmp = shp.tile([P, d], DT, tag=nm, name=nm + f"t{tt}")
        nc.vector.tensor_tensor(out=tmp[:], in0=shps[:], in1=buf[:, tt, :],
                                op=Alu.subtract)
        nc.gpsimd.tensor_tensor(out=tmp[:], in0=tmp[:], in1=coef[:],
                                op=Alu.mult)
        nc.gpsimd.tensor_tensor(out=buf[:, tt, :], in0=buf[:, tt, :],
                                in1=tmp[:], op=Alu.add)
        st = shp.tile([P, 6], f32, tag=nm + "st", name=nm + f"s{tt}")
        nc.vector.bn_stats(out=st[:], in_=buf[:, tt, :])
        nc.vector.bn_aggr(out=statbuf[:, :, tt:tt + 1], in_=st[:])

    def norm_apply(buf, n, statbuf, g, b):
        nc.scalar.activation(statbuf[:, 1, :], statbuf[:, 1, :], Act.Sqrt,
                             bias=eps_sb[:])
        nc.vector.reciprocal(statbuf[:, 1, :], statbuf[:, 1, :])
        for tt in range(n):
            nc.vector.tensor_scalar(out=buf[:, tt, :], in0=buf[:, tt, :],
                                    scalar1=statbuf[:, 0, tt:tt + 1],
                                    scalar2=statbuf[:, 1, tt:tt + 1],
                                    op0=Alu.subtract, op1=Alu.mult)
            if g is not None:
                nc.gpsimd.tensor_tensor(out=buf[:, tt, :], in0=buf[:, tt, :],
                                        in1=g[:], op=Alu.mult)
                nc.gpsimd.tensor_tensor(out=buf[:, tt, :], in0=buf[:, tt, :],
                                        in1=b[:], op=Alu.add)

    with (tc.tile_pool(name="shp", bufs=3) as shp,
          tc.tile_pool(name="ps_c", bufs=2, space="PSUM") as ps_c):
        for tt in range(NT - 1, -1, -1):
            prev = None if tt == 0 else (bnd, kk[:, tt - 1, :])
            blend_stats(kk, tt, QD, xk_rep, "ksh", prev, statk)
        norm_apply(kk, NT, statk, gk, bk)
        for tt in range(NQT - 1, -1, -1):
            prev = ((qsel[tt // 2], qprev) if tt % 2 == 0
                    else (bnd, qraw[:, tt - 1, :]))
            blend_stats(qraw, tt, QD, xq_rep, "qsh", prev, statq)
        norm_apply(qraw, NQT, statq, gq, bq)

    # transposes into attention layouts
    with tc.tile_pool(name="ps_t", bufs=2, space="PSUM") as ps_t:
        for tt in range(NT):
            for qc in range(2):
                tps = ps_t.tile([P, P], DT, tag="tps")
                nc.tensor.transpose(tps[:], kk[:, tt, qc * P:(qc + 1) * P],
                                    ident[:])
                nc.vector.tensor_copy(out=kT[:, qc, tt * P:(tt + 1) * P],
                                      in_=tps[:])
        for tt in range(NQT):
            for qc in range(2):
                tps = ps_t.tile([P, P], DT, tag="tps")
                nc.tensor.transpose(tps[:], qraw[:, tt, qc * P:(qc + 1) * P],
                                    ident[:])
                nc.vector.tensor_copy(out=qT[:, qc, tt * P:(tt + 1) * P],
                                      in_=tps[:])

    # ---------------- Phase D: attention ----------------
    with (tc.tile_pool(name="att", bufs=4) as attp,
          tc.tile_pool(name="outs", bufs=2) as outsp,
          tc.tile_pool(name="ps_sc", bufs=2, space="PSUM") as ps_sc,
          tc.tile_pool(name="ps_out", bufs=1, space="PSUM") as ps_out,
          tc.tile_pool(name="ps_sum", bufs=1, space="PSUM") as ps_sum):
        for s in range(NSLOT):
            sums = [ps_sum.tile([P, 1], f32, tag=f"sums{i}",
                                name=f"sums_{s}_{i}") for i in range(2)]
            ops = [ps_out.tile([P, 512], f32, tag=f"o{i}{ch}",
                               name=f"ops_{s}_{i}{ch}")
                   for i in range(2) for ch in range(2)]
            # k-tiles processed in pairs: one [128, 2, 256] score block per
            # pair halves the tanh/exp/mask op count. Mask-needing k-tiles
            # are pair-aligned per slot by construction.
            for kp in range(R[s] // 2):
                sps = ps_sc.tile([P, 2, CHUNK], f32, tag="sps")
                for h in range(2):
                    kt = 2 * kp + h
                    for qc in range(2):
                        nc.tensor.matmul(
                            sps[:, h, :], kT[:, qc, kt * P:(kt + 1) * P],
                            qT[:, qc, s * CHUNK:(s + 1) * CHUNK],
                            start=(qc == 0), stop=(qc == 1))
                et = attp.tile([P, 2, CHUNK], DT, tag="et")
                nc.scalar.activation(et[:], sps[:], Act.Tanh,
                                     scale=1.0 / SCORE_SCALE)
                if (s, 2 * kp) in MASK_IDX:
                    mi = MASK_IDX[(s, 2 * kp)]
                    assert MASK_IDX[(s, 2 * kp + 1)] == mi + 1
                    nc.gpsimd.tensor_tensor(
                        out=et[:], in0=et[:],
                        in1=maskall[:, mi:mi + 2, :], op=Alu.add)
                ee = attp.tile([P, 2, CHUNK], DT, tag="ee")
                nc.scalar.activation(ee[:], et[:], Act.Exp, scale=CAP_SCALE)
                for h in range(2):
                    kt = 2 * kp + h
                    first, last = kt == 0, kt == R[s] - 1
                    for i in range(2):
                        nc.tensor.matmul(sums[i][:],
                                         ee[:, h, i * P:(i + 1) * P],
                                         ones1[:], start=first, stop=last)
                        for ch in range(2):
                            nc.tensor.matmul(
                                ops[2 * i + ch][:],
                                ee[:, h, i * P:(i + 1) * P],
                                vv[:, kt, ch * 512:(ch + 1) * 512],
                                start=first, stop=last)
            recip = attp.tile([P, 2], f32, tag="recip")
            for i in range(2):
                nc.vector.reciprocal(recip[:, i:i + 1], sums[i][:])
            for i in range(2):
                ot = outsp.tile([P, C], mybir.dt.float32, tag="ot")
                for ch in range(2):
                    nc.vector.tensor_scalar_mul(
                        out=ot[:, ch * 512:(ch + 1) * 512],
                        in0=ops[2 * i + ch][:], scalar1=recip[:, i:i + 1])
                nc.scalar.dma_start(
                    out_d[s * CHUNK + i * P:s * CHUNK + (i + 1) * P, :],
                    ot[:])

    if loop is not None:
        loop.__exit__(None, None, None)
    ctx.close()


_NC_CACHE = {}


def _input_specs(apply_gb, bf16):
    import concourse.mybir as mybir
    f32 = mybir.dt.float32
    DT = mybir.dt.bfloat16 if bf16 else f32
    specs = [
        ("xT", [C, T], DT), ("xqT", [C, TQ], DT),
        ("xqprevT", [C, NSLOT], DT),
        ("kemb", [T, QD], DT), ("vemb1", [T, C], DT),
        ("vemb2", [T, C], DT),
        ("wqq", [C, QD], DT), ("wkv", [C, 64], DT),
        ("wkup", [KV, QD], DT), ("wvup", [KV, C], DT),
        ("xq_rep", [P, QD], DT), ("xk_rep", [P, QD], DT),
        ("xv_rep", [P, C], DT),
        ("mask", [NMASK, P, CHUNK], DT),
    ]
    if apply_gb:
        specs += [("gq_rep", [P, QD], DT), ("bq_rep", [P, QD], DT),
                  ("gk_rep", [P, QD], DT), ("bk_rep", [P, QD], DT),
                  ("gv_rep", [P, C], DT), ("bv_rep", [P, C], DT)]
    return specs


def get_nc(apply_gb, bf16=True, nrep=1):
    key = (bool(apply_gb), bool(bf16), int(nrep))
    if key in _NC_CACHE:
        return _NC_CACHE[key]
    import concourse.mybir as mybir
    import concourse.tile as tile
    from concourse import bacc

    nc = bacc.Bacc("TRN2", target_bir_lowering=False, debug=False,
                   num_devices=N_CORES)
    a = {}
    for name, shape, dt in _input_specs(apply_gb, bf16):
        a[name] = nc.dram_tensor(name, shape, dt, kind="ExternalInput").ap()
    a["out"] = nc.dram_tensor("out", [TQ, C], mybir.dt.float32,
                              kind="ExternalOutput").ap()
    with tile.TileContext(nc) as tc:
        _build_program(nc, tc, a, apply_gb, bf16, nrep=nrep)
    nc.compile()
    _NC_CACHE[key] = nc
    return nc


def _parity_mask(parity):
    m = np.zeros((NMASK, P, CHUNK), np.float32)
    for (s, kt), mi in MASK_IDX.items():
        qs = CHUNKS[parity][s] * CHUNK
        kg = np.arange(P, dtype=np.int64)[:, None] + P * kt
        qg = np.arange(CHUNK, dtype=np.int64)[None, :] + qs
        m[mi] = np.where(qg >= kg, 0.0, NEG).astype(np.float32)
    return m


def make_in_maps(inputs, bf16=True):
    import ml_dtypes
    cdt = ml_dtypes.bfloat16 if bf16 else np.float32

    x = np.asarray(inputs["x"], np.float32)
    idx = np.asarray(inputs["idx"]).astype(np.int64)
    k_tab = np.asarray(inputs["k_emb_tab"], np.float32)
    v_tab = np.asarray(inputs["v_emb_tab"], np.float32)
    W_qq = np.asarray(inputs["W_qq"], np.float32)
    W_k = np.asarray(inputs["W_k"], np.float32)
    W_kup = np.asarray(inputs["W_kup"], np.float32)
    W_v = np.asarray(inputs["W_v"], np.float32)
    W_vup = np.asarray(inputs["W_vup"], np.float32)
    x_q = np.asarray(inputs["x_q"], np.float32).reshape(QD)
    x_k = np.asarray(inputs["x_k"], np.float32).reshape(QD)
    x_v = np.asarray(inputs["x_v"], np.float32).reshape(C)
    g_q = np.asarray(inputs["g_q"], np.float32).reshape(QD)
    b_q = np.asarray(inputs["b_q"], np.float32).reshape(QD)
    g_k = np.asarray(inputs["g_k"], np.float32).reshape(QD)
    b_k = np.asarray(inputs["b_k"], np.float32).reshape(QD)
    g_v = np.asarray(inputs["g_v"], np.float32).reshape(C)
    b_v = np.asarray(inputs["b_v"], np.float32).reshape(C)

    apply_gb = not (np.all(g_q == 1) and np.all(b_q == 0)
                    and np.all(g_k == 1) and np.all(b_k == 0)
                    and np.all(g_v == 1) and np.all(b_v == 0))

    k_emb = k_tab[idx]          # [B, T, QD]
    v_emb = v_tab[idx]          # [B, T, C]
    vemb1 = [np.ascontiguousarray(v_emb[b] * (1.0 - x_v)).astype(cdt)
             for b in range(B)]
    vemb2 = [np.ascontiguousarray(v_emb[b] * x_v).astype(cdt)
             for b in range(B)]

    def cvt(arr):
        return np.ascontiguousarray(arr).astype(cdt)

    shared = {
        "wqq": cvt(W_qq.T),
        "wkv": cvt(np.concatenate([W_k, W_v], 0).T),
        "wkup": cvt(W_kup.T),
        "wvup": cvt(W_vup.T),
        "xq_rep": cvt(np.broadcast_to(x_q, (P, QD))),
        "xk_rep": cvt(np.broadcast_to(x_k, (P, QD))),
        "xv_rep": cvt(np.broadcast_to(x_v, (P, C))),
    }
    if apply_gb:
        for nm, v in [("gq", g_q), ("bq", b_q), ("gk", g_k), ("bk", b_k)]:
            shared[nm + "_rep"] = cvt(np.broadcast_to(v, (P, QD)))
        for nm, v in [("gv", g_v), ("bv", b_v)]:
            shared[nm + "_rep"] = cvt(np.broadcast_to(v, (P, C)))

    pmask = [_parity_mask(0).astype(cdt), _parity_mask(1).astype(cdt)]
    in_maps = []
    for c in range(N_CORES):
        b, parity = c // 2, c % 2
        chunks = CHUNKS[parity]
        cols = np.concatenate([np.arange(ch * CHUNK, (ch + 1) * CHUNK)
                               for ch in chunks])
        xqprev = np.zeros((NSLOT, C), np.float32)
        for j, ch in enumerate(chunks):
            if ch > 0:
                xqprev[j] = x[b, ch * CHUNK - 1]
        m = dict(shared)
        m.update(
            xT=cvt(x[b].T), xqT=cvt(x[b][cols].T),
            xqprevT=cvt(xqprev.T),
            kemb=cvt(k_emb[b]),
            vemb1=vemb1[b], vemb2=vemb2[b],
            mask=pmask[parity],
        )
        in_maps.append(m)
    return in_maps, apply_gb


def assemble_output(results):
    out = np.empty((B, T, C), np.float32)
    for c in range(N_CORES):
        oc = results[c]["out"]
        for j, ch in enumerate(CHUNKS[c % 2]):
            out[c // 2, ch * CHUNK:(ch + 1) * CHUNK] = \
                oc[j * CHUNK:(j + 1) * CHUNK]
    return out


BF16 = True


def kernel(**inputs):
    from concourse.bass_utils import run_bass_kernel_spmd
    in_maps, apply_gb = make_in_maps(inputs, bf16=BF16)
    nc = get_nc(apply_gb, bf16=BF16)
    res = run_bass_kernel_spmd(nc, in_maps, core_ids=list(range(N_CORES)))
    return assemble_output(res.results)


# revision 39
# speedup vs baseline: 339.4980x; 10.6058x over previous
"""DeepEmbedAttention TRN2 kernel — 8-core SPMD.

Sharding: 2 cores per batch (B=4). Each core computes the full k/v chain for
its batch (T=2048) and attention outputs for 4 query chunks of 256 tokens.
Chunk assignment is causally load-balanced: even cores take chunks {0,3,4,7},
odd cores {1,2,5,6}. The single SPMD program processes chunks at canonical
slot positions; everything position-dependent (q columns, causal masks,
chunk-boundary tokens) arrives as per-core input data, so one program serves
all 8 cores. Softmax needs no max-subtraction: scores are tanh-capped to
[-64, 64], so exp() cannot overflow fp32.

Engine plan: token-shift is done with PE matmuls against constant
superdiagonal/boundary selector matrices (DMA-free). DMAs are batched large
and spread over the three issuing queues (sync = input streams, scalar =
output stores, gpsimd = constants); SBUF-only elementwise work is offloaded
to the otherwise-idle GpSimd engine.
"""

import sys

if "/opt/trn_rl_repo" not in sys.path:
    sys.path.insert(0, "/opt/trn_rl_repo")

import numpy as np

B, T, C = 4, 2048, 1024
QD, KV = 256, 32
SCORE_SCALE, CAP_SCALE = 1024.0, 64.0
EPS = 1e-5
N_CORES = 8
P = 128
CHUNK = 256
NSLOT = 4                       # q-chunks per core
TQ = NSLOT * CHUNK              # 1024 canonical query tokens per core
NT = T // P                     # 16 token tiles (full sequence)
NQT = TQ // P                   # 8 canonical query token tiles
CHUNKS = [[0, 3, 4, 7], [1, 2, 5, 6]]   # parity -> global chunk ids
R = [4, 8, 12, 16]              # k-tiles per slot (max over parities)
MINQS = [0, 512, 1024, 1536]    # min chunk start over parities, per slot
NEED_MASK = [(s, kt) for s in range(NSLOT) for kt in range(R[s])
             if P * (kt + 1) > MINQS[s]]
MASK_IDX = {sk: i for i, sk in enumerate(NEED_MASK)}
NMASK = len(NEED_MASK)          # 16
NEG = -1.0e30


def _build_program(nc, tc, a, apply_gb, bf16, nrep=1, phases=4):
    from contextlib import ExitStack

    import concourse.mybir as mybir
    from concourse.masks import make_identity

    f32 = mybir.dt.float32
    DT = mybir.dt.bfloat16 if bf16 else f32
    NMAX = 512                      # psum-bank limit caps matmul free size
    Alu = mybir.AluOpType
    Act = mybir.ActivationFunctionType

    xTr = a["xT"].rearrange("(a p) t -> p a t", p=P)        # [128, 8, 2048]
    xqTr = a["xqT"].rearrange("(a p) t -> p a t", p=P)      # [128, 8, 1024]
    xqpr = a["xqprevT"].rearrange("(a p) t -> p a t", p=P)  # [128, 8, 4]
    wqqr = a["wqq"].rearrange("(a p) d -> p a d", p=P)      # [128, 8, 256]
    wkvr = a["wkv"].rearrange("(a p) d -> p a d", p=P)      # [128, 8, 64]
    kembr = a["kemb"].rearrange("(g p) d -> p g d", p=P)    # [128, 16, 256]
    vembr1 = a["vemb1"].rearrange("(g p) d -> p g d", p=P)  # [128, 16, 1024]
    vembr2 = a["vemb2"].rearrange("(g p) d -> p g d", p=P)
    maskr = a["mask"].rearrange("m p q -> p m q")           # [128, 16, 256]
    out_d = a["out"]                                        # [1024, 1024]

    ctx = ExitStack()
    const = ctx.enter_context(tc.tile_pool(name="const", bufs=1))
    pers = ctx.enter_context(tc.tile_pool(name="pers", bufs=1))

    # --- constants (gpsimd queue for the DMAs) ---
    ident = const.tile([P, P], DT, tag="ident")
    make_identity(nc, ident[:])
    # ssup[p, m] = 1 iff m == p+1 : shift-down-one (sh[m] = v[m-1])
    ssup = const.tile([P, P], DT, tag="ssup")
    nc.gpsimd.memset(ssup[:], 0.0)
    nc.gpsimd.affine_select(out=ssup[:], in_=ssup[:],
                            compare_op=Alu.not_equal, fill=1.0,
                            base=1, pattern=[[-1, P]], channel_multiplier=1)
    # bnd[p, m] = 1 iff (p==127, m==0) : carry prev tile's last row into row 0
    bnd = const.tile([P, P], DT, tag="bnd")
    nc.gpsimd.memset(bnd[:], 0.0)
    nc.gpsimd.affine_select(out=bnd[:], in_=bnd[:],
                            compare_op=Alu.not_equal, fill=1.0,
                            base=-(P - 1), pattern=[[-P, P]],
                            channel_multiplier=1)
    # qsel[s][p, m] = 1 iff (p==s, m==0) : qprev row s into row 0
    qsel = []
    for s in range(NSLOT):
        qs_t = const.tile([NSLOT, P], DT, tag=f"qsel{s}", name=f"qsel{s}")
        nc.gpsimd.memset(qs_t[:], 0.0)
        nc.gpsimd.affine_select(out=qs_t[:], in_=qs_t[:],
                                compare_op=Alu.not_equal, fill=1.0,
                                base=-s, pattern=[[-NSLOT, P]],
                                channel_multiplier=1)
        qsel.append(qs_t)
    ones1 = const.tile([P, 1], DT, tag="ones1")
    nc.gpsimd.memset(ones1[:], 1.0)
    eps_sb = const.tile([P, 1], f32, tag="eps")
    nc.gpsimd.memset(eps_sb[:], EPS)

    wkup = const.tile([KV, QD], DT, tag="wkup")
    nc.gpsimd.dma_start(wkup[:], a["wkup"][:])
    # v_mid lives at base partition 32 inside kvmid; PE needs lhsT/rhs bases
    # to match, so W_vupT is loaded at partitions 32..63 as well.
    wvup64 = const.tile([64, C], DT, tag="wvup")
    nc.gpsimd.dma_start(wvup64[KV:64, :], a["wvup"][:])
    wvup = wvup64[KV:64, :]
    wqq = const.tile([P, 8, QD], DT, tag="wqq")
    nc.gpsimd.dma_start(wqq[:], wqqr[:])
    wkv = const.tile([P, 8, 64], DT, tag="wkv")
    nc.gpsimd.dma_start(wkv[:], wkvr[:])
    xq_rep = const.tile([P, QD], DT, tag="xq_rep")
    nc.gpsimd.dma_start(xq_rep[:], a["xq_rep"][:])
    xk_rep = const.tile([P, QD], DT, tag="xk_rep")
    nc.gpsimd.dma_start(xk_rep[:], a["xk_rep"][:])
    xv_rep = const.tile([P, C], DT, tag="xv_rep")
    nc.gpsimd.dma_start(xv_rep[:], a["xv_rep"][:])
    maskall = const.tile([P, NMASK, CHUNK], DT, tag="maskall")
    nc.gpsimd.dma_start(maskall[:], maskr[:])
    gb = {}
    if apply_gb:
        for nm, d in [("gq", QD), ("bq", QD), ("gk", QD), ("bk", QD),
                      ("gv", C), ("bv", C)]:
            gb[nm] = const.tile([P, d], DT, tag=nm + "_rep", name=nm + "_rep")
            nc.gpsimd.dma_start(gb[nm][:], a[nm + "_rep"][:])

    loop = tc.For_i(0, nrep, 1) if nrep > 1 else None
    if loop is not None:
        loop.__enter__()

    # --- persistent strips ---
    kvmid = pers.tile([64, T], DT, tag="kvmid")       # [k_mid; v_mid]^T
    qraw = pers.tile([P, NQT, QD], DT, tag="qraw")    # canonical q tiles
    qprev = pers.tile([NSLOT, QD], DT, tag="qprev")   # chunk-boundary q rows
    kk = pers.tile([P, NT, QD], DT, tag="kk")         # k chain, [T, QD] tiles
    vv = pers.tile([P, NT, C], DT, tag="vv")          # v chain, [T, C] tiles
    kT = pers.tile([P, 2, T], DT, tag="kT")           # k^T for attention
    qT = pers.tile([P, 2, TQ], DT, tag="qT")          # q^T for attention

    gq, bq = (gb.get("gq"), gb.get("bq"))
    gk, bk = (gb.get("gk"), gb.get("bk"))
    gv, bv = (gb.get("gv"), gb.get("bv"))

    i32 = mybir.dt.int32

    def rsqrt_dve(x_ap, w, pool, nm):
        # In-place x <- rsqrt(x + EPS) entirely on DVE: magic-constant seed
        # + 2 Newton iterations. Keeps Sqrt off ACT, whose function table
        # would need a ~1.3us reload to switch away from tanh/exp/square.
        nc.vector.tensor_scalar_add(out=x_ap, in0=x_ap, scalar1=EPS)
        yi = pool.tile([P, w], i32, tag=nm + "yi", name=nm + "yi")
        nc.vector.tensor_scalar(out=yi[:], in0=x_ap.bitcast(i32),
                                scalar1=1, scalar2=None,
                                op0=Alu.arith_shift_right)
        nc.vector.tensor_scalar(out=yi[:], in0=yi[:], scalar1=-1,
                                scalar2=0x5F3759DF, op0=Alu.mult,
                                op1=Alu.add)
        y = yi[:].bitcast(f32)
        t2 = pool.tile([P, w], f32, tag=nm + "t2", name=nm + "t2")
        for _ in range(2):
            nc.vector.tensor_tensor(out=t2[:], in0=y, in1=y, op=Alu.mult)
            nc.vector.tensor_tensor(out=t2[:], in0=t2[:], in1=x_ap,
                                    op=Alu.mult)
            nc.vector.tensor_scalar(out=t2[:], in0=t2[:], scalar1=-0.5,
                                    scalar2=1.5, op0=Alu.mult, op1=Alu.add)
            nc.vector.tensor_tensor(out=y, in0=y, in1=t2[:], op=Alu.mult)
        nc.vector.tensor_copy(out=x_ap, in_=y)

    # ---------------- Phase A: kv_mid + q projections ----------------
    with (tc.tile_pool(name="xin", bufs=(4 if bf16 else 2)) as xin,
          tc.tile_pool(name="ps_a", bufs=2, space="PSUM") as ps_a):
        for tb in range(T // 512):
            xt = xin.tile([P, 8, 512], DT, tag="xt")
            nc.sync.dma_start(xt[:], xTr[:, :, tb * 512:(tb + 1) * 512])
            kvps = ps_a.tile([64, 512], f32, tag="kvps")
            for cc in range(8):
                nc.tensor.matmul(kvps[:], wkv[:, cc, :], xt[:, cc, :],
                                 start=(cc == 0), stop=(cc == 7))
            nc.scalar.copy(kvmid[:, tb * 512:(tb + 1) * 512], kvps[:])

        for th in range(2):     # canonical q in two 512-token halves
            xqt = xin.tile([P, 8, 512], DT, tag="xt", name="xqt")
            nc.sync.dma_start(xqt[:], xqTr[:, :, th * 512:(th + 1) * 512])
            for j in range(4):
                tt = th * 4 + j
                qps = ps_a.tile([P, QD], f32, tag="qps")
                for cc in range(8):
                    nc.tensor.matmul(qps[:], xqt[:, cc, j * P:(j + 1) * P],
                                     wqq[:, cc, :],
                                     start=(cc == 0), stop=(cc == 7))
                nc.scalar.copy(qraw[:, tt, :], qps[:])

        xqp = xin.tile([P, 8, NSLOT], DT, tag="xqp")
        nc.sync.dma_start(xqp[:], xqpr[:])
        qpps = ps_a.tile([NSLOT, QD], f32, tag="qpps")
        for cc in range(8):
            nc.tensor.matmul(qpps[:], xqp[:, cc, :], wqq[:, cc, :],
                             start=(cc == 0), stop=(cc == 7))
        nc.scalar.copy(qprev[:], qpps[:])

    # ---------------- Phase B: k up-proj + embeddings ----------------
    if phases < 2:
        if loop is not None:
            loop.__exit__(None, None, None)
        ctx.close()
        return
    with (tc.tile_pool(name="emb", bufs=3) as embp,
          tc.tile_pool(name="ps_b", bufs=2, space="PSUM") as ps_b):
        for g in range(NT // 4):
            kemb = embp.tile([P, 4, QD], DT, tag="kemb")
            nc.scalar.dma_start(kemb[:], kembr[:, g * 4:(g + 1) * 4, :])
            for j in range(4):
                tt = g * 4 + j
                kps = ps_b.tile([P, QD], f32, tag="kps")
                nc.tensor.matmul(kps[:], kvmid[0:KV, tt * P:(tt + 1) * P],
                                 wkup[:], start=True, stop=True)
                nc.vector.tensor_tensor(out=kk[:, tt, :], in0=kps[:],
                                        in1=kemb[:, j, :], op=Alu.mult)

    # ------- v chain: up-proj + pre-scaled embeddings + shift + LN -------
    # vemb1 = v_emb*(1-x_v), vemb2 = v_emb*x_v (host-prescaled), so
    # blended = tanh(proj)*vemb1 + S(tanh(proj)*vemb2): the blend collapses
    # into two SBUF multiplies plus the shift matmul, no sub/mul/add chain.
    # Ascending order; v2 of the previous tile feeds the boundary matmul.
    with (tc.tile_pool(name="vemb", bufs=2) as vembp,
          tc.tile_pool(name="vwork", bufs=4) as vwork,
          tc.tile_pool(name="ps_v", bufs=2, space="PSUM") as ps_v):
        v2_prev = None
        for tt in range(NT):
            g, j = tt // 4, tt % 4
            if j == 0:
                vemb1 = vembp.tile([P, 4, C], DT, tag="vemb1",
                                   name=f"vemb1_{g}")
                nc.scalar.dma_start(vemb1[:],
                                    vembr1[:, g * 4:(g + 1) * 4, :])
                vemb2 = vembp.tile([P, 4, C], DT, tag="vemb2",
                                   name=f"vemb2_{g}")
                nc.scalar.dma_start(vemb2[:],
                                    vembr2[:, g * 4:(g + 1) * 4, :])
            vps = ps_v.tile([P, C], f32, tag="vps")
            for ch in range(2):
                nc.tensor.matmul(vps[:, ch * 512:(ch + 1) * 512],
                                 kvmid[KV:64, tt * P:(tt + 1) * P],
                                 wvup[:, ch * 512:(ch + 1) * 512],
                                 start=True, stop=True)
            vt = vwork.tile([P, C], DT, tag="vt", name=f"vt{tt}")
            nc.scalar.activation(vt[:], vps[:], Act.Tanh)
            v2 = vwork.tile([P, C], DT, tag="v2", name=f"v2_{tt}")
            nc.vector.tensor_tensor(out=vv[:, tt, :], in0=vt[:],
                                    in1=vemb1[:, j, :], op=Alu.mult)
            nc.vector.tensor_tensor(out=v2[:], in0=vt[:],
                                    in1=vemb2[:, j, :], op=Alu.mult)
            shps = ps_v.tile([P, C], f32, tag="shps", name=f"vsh{tt}")
            for ch in range(0, C, NMAX):
                ce = ch + NMAX
                nc.tensor.matmul(shps[:, ch:ce], ssup[:], v2[:, ch:ce],
                                 start=True, stop=v2_prev is None)
                if v2_prev is not None:
                    nc.tensor.matmul(shps[:, ch:ce], bnd[:],
                                     v2_prev[:, ch:ce],
                                     start=False, stop=True)
            nc.vector.tensor_tensor(out=vv[:, tt, :], in0=shps[:],
                                    in1=vv[:, tt, :], op=Alu.add)
            v2_prev = v2
            # LN stats on ACT via accum_out (frees DVE's bn_stats). Sqrt is
            # deferred and batched: it lives in a different ACT function
            # table than Tanh/Exp/Square/Copy, and per-tile use would force
            # a ~1.3us table reload per switch.
            scr = vwork.tile([P, C], DT, tag="vscr", name=f"vscr{tt}")
            ssq = vwork.tile([P, 1], f32, tag="vssq", name=f"vssq{tt}")
            nc.scalar.activation(scr[:], vv[:, tt, :], Act.Square,
                                 accum_out=ssq[:])
            ssm = vwork.tile([P, 1], f32, tag="vssm", name=f"vssm{tt}")
            nc.scalar.activation(scr[:], vv[:, tt, :], Act.Copy,
                                 accum_out=ssm[:])
            mv = vwork.tile([P, 2], f32, tag="vmv", name=f"vmv{tt}")
            nc.vector.tensor_scalar_mul(out=mv[:, 0:1],
                                        in0=ssm[:], scalar1=1.0 / C)
            msq = vwork.tile([P, 1], f32, tag="vmsq", name=f"vmsq{tt}")
            nc.vector.tensor_tensor(out=msq[:], in0=mv[:, 0:1],
                                    in1=mv[:, 0:1], op=Alu.mult)
            nc.vector.tensor_scalar_mul(out=mv[:, 1:2],
                                        in0=ssq[:], scalar1=1.0 / C)
            nc.vector.tensor_tensor(out=mv[:, 1:2], in0=mv[:, 1:2],
                                    in1=msq[:], op=Alu.subtract)
            rsqrt_dve(mv[:, 1:2], 1, vwork, "vrs")
            nc.vector.tensor_scalar(out=vv[:, tt, :], in0=vv[:, tt, :],
                                    scalar1=mv[:, 0:1],
                                    scalar2=mv[:, 1:2],
                                    op0=Alu.subtract, op1=Alu.mult)
            if gv is not None:
                nc.gpsimd.tensor_tensor(out=vv[:, tt, :], in0=vv[:, tt, :],
                                        in1=gv[:], op=Alu.mult)
                nc.gpsimd.tensor_tensor(out=vv[:, tt, :], in0=vv[:, tt, :],
                                        in1=bv[:], op=Alu.add)

    # ---------------- Phase C: shift + blend + layernorm ----------------
    if phases < 3:
        if loop is not None:
            loop.__exit__(None, None, None)
        ctx.close()
        return
    # Token shift via PE: sh = ssup.T @ tile (+ bnd.T @ prev_tile). The blend
    # writes a separate destination tile (src strip stays pre-blend), so the
    # chains run ASCENDING and per-tile normalize + transpose follow
    # immediately — attention's dependencies resolve at slot granularity.
    def blend_ln_t(src, dst, dstT, tt, coef, nm, prev_rhs, g, b):
        shps = ps_c.tile([P, QD], f32, tag="shps", name=nm + f"ps{tt}")
        nc.tensor.matmul(shps[:], ssup[:], src[:, tt, :],
                         start=True, stop=prev_rhs is None)
        if prev_rhs is not None:
            nc.tensor.matmul(shps[:], prev_rhs[0], prev_rhs[1],
                             start=False, stop=True)
        # gpsimd (Pool) takes the SBUF-only blend ops; the psum-reading
        # subtract stays on DVE.
        tmp = shp.tile([P, QD], DT, tag=nm, name=nm + f"t{tt}")
        nc.vector.tensor_tensor(out=tmp[:], in0=shps[:], in1=src[:, tt, :],
                                op=Alu.subtract)
        nc.gpsimd.tensor_tensor(out=tmp[:], in0=tmp[:], in1=coef[:],
                                op=Alu.mult)
        nc.gpsimd.tensor_tensor(out=dst[:, tt, :], in0=src[:, tt, :],
                                in1=tmp[:], op=Alu.add)
        st = shp.tile([P, 6], f32, tag=nm + "st", name=nm + f"s{tt}")
        nc.vector.bn_stats(out=st[:], in_=dst[:, tt, :])
        mv = shp.tile([P, 2], f32, tag=nm + "mv", name=nm + f"m{tt}")
        nc.vector.bn_aggr(out=mv[:], in_=st[:])
        rsqrt_dve(mv[:, 1:2], 1, shp, nm + "rs")
        nc.vector.tensor_scalar(out=dst[:, tt, :], in0=dst[:, tt, :],
                                scalar1=mv[:, 0:1], scalar2=mv[:, 1:2],
                                op0=Alu.subtract, op1=Alu.mult)
        if g is not None:
            nc.gpsimd.tensor_tensor(out=dst[:, tt, :], in0=dst[:, tt, :],
                                    in1=g[:], op=Alu.mult)
            nc.gpsimd.tensor_tensor(out=dst[:, tt, :], in0=dst[:, tt, :],
                                    in1=b[:], op=Alu.add)
        for qc in range(2):
            tps = ps_c.tile([P, P], DT, tag="tps")
            nc.tensor.transpose(tps[:], dst[:, tt, qc * P:(qc + 1) * P],
                                ident[:])
            nc.vector.tensor_copy(out=dstT[:, qc, tt * P:(tt + 1) * P],
                                  in_=tps[:])

    kf = pers.tile([P, NT, QD], DT, tag="kf")
    qf = pers.tile([P, NQT, QD], DT, tag="qf")
    with (tc.tile_pool(name="shp", bufs=4) as shp,
          tc.tile_pool(name="ps_c", bufs=3, space="PSUM") as ps_c):
        for tt in range(NQT):
            prev = ((qsel[tt // 2], qprev[:]) if tt % 2 == 0
                    else (bnd, qraw[:, tt - 1, :]))
            blend_ln_t(qraw, qf, qT, tt, xq_rep, "qsh", prev, gq, bq)
        for tt in range(NT):
            prev = None if tt == 0 else (bnd, kk[:, tt - 1, :])
            blend_ln_t(kk, kf, kT, tt, xk_rep, "ksh", prev, gk, bk)

    # ---------------- Phase D: attention ----------------
    if phases < 4:
        if loop is not None:
            loop.__exit__(None, None, None)
        ctx.close()
        return
    with (tc.tile_pool(name="att", bufs=6) as attp,
          tc.tile_pool(name="outs", bufs=2) as outsp,
          tc.tile_pool(name="ps_sc", bufs=2, space="PSUM") as ps_sc,
          tc.tile_pool(name="ps_out", bufs=1, space="PSUM") as ps_out,
          tc.tile_pool(name="ps_sum", bufs=1, space="PSUM") as ps_sum):
        for s in range(NSLOT):
            sums = [ps_sum.tile([P, 1], f32, tag=f"sums{i}",
                                name=f"sums_{s}_{i}") for i in range(2)]
            ops = [ps_out.tile([P, 512], f32, tag=f"o{i}{ch}",
                               name=f"ops_{s}_{i}{ch}")
                   for i in range(2) for ch in range(2)]
            # k-tiles processed in pairs: one [128, 2, 256] score block per
            # pair halves the tanh/exp/mask op count. Mask-needing k-tiles
            # are pair-aligned per slot by construction.
            for kp in range(R[s] // 2):
                sps = ps_sc.tile([P, 2, CHUNK], f32, tag="sps")
                for h in range(2):
                    kt = 2 * kp + h
                    for qc in range(2):
                        nc.tensor.matmul(
                            sps[:, h, :], kT[:, qc, kt * P:(kt + 1) * P],
                            qT[:, qc, s * CHUNK:(s + 1) * CHUNK],
                            start=(qc == 0), stop=(qc == 1))
                et = attp.tile([P, 2, CHUNK], DT, tag="et")
                nc.scalar.activation(et[:], sps[:], Act.Tanh,
                                     scale=1.0 / SCORE_SCALE)
                if (s, 2 * kp) in MASK_IDX:
                    mi = MASK_IDX[(s, 2 * kp)]
                    assert MASK_IDX[(s, 2 * kp + 1)] == mi + 1
                    nc.gpsimd.tensor_tensor(
                        out=et[:], in0=et[:],
                        in1=maskall[:, mi:mi + 2, :], op=Alu.add)
                ee = attp.tile([P, 2, CHUNK], DT, tag="ee")
                nc.scalar.activation(ee[:], et[:], Act.Exp, scale=CAP_SCALE)
                for h in range(2):
                    kt = 2 * kp + h
                    first, last = kt == 0, kt == R[s] - 1
                    for i in range(2):
                        nc.tensor.matmul(sums[i][:],
                                         ee[:, h, i * P:(i + 1) * P],
                                         ones1[:], start=first, stop=last)
                        for ch in range(2):
                            nc.tensor.matmul(
                                ops[2 * i + ch][:],
                                ee[:, h, i * P:(i + 1) * P],
                                vv[:, kt, ch * 512:(ch + 1) * 512],
                                start=first, stop=last)
            recip = attp.tile([P, 2], f32, tag="recip")
            for i in range(2):
                nc.vector.reciprocal(recip[:, i:i + 1], sums[i][:])
            for i in range(2):
                ot = outsp.tile([P, C], mybir.dt.float32, tag="ot")
                for ch in range(2):
                    nc.vector.tensor_scalar_mul(
                        out=ot[:, ch * 512:(ch + 1) * 512],
                        in0=ops[2 * i + ch][:], scalar1=recip[:, i:i + 1])
                nc.scalar.dma_start(
                    out_d[s * CHUNK + i * P:s * CHUNK + (i + 1) * P, :],
                    ot[:])

    if loop is not None:
        loop.__exit__(None, None, None)
    ctx.close()


_NC_CACHE = {}


def _input_specs(apply_gb, bf16):
    import concourse.mybir as mybir
    f32 = mybir.dt.float32
    DT = mybir.dt.bfloat16 if bf16 else f32
    specs = [
        ("xT", [C, T], DT), ("xqT", [C, TQ], DT),
        ("xqprevT", [C, NSLOT], DT),
        ("kemb", [T, QD], DT), ("vemb1", [T, C], DT),
        ("vemb2", [T, C], DT),
        ("wqq", [C, QD], DT), ("wkv", [C, 64], DT),
        ("wkup", [KV, QD], DT), ("wvup", [KV, C], DT),
        ("xq_rep", [P, QD], DT), ("xk_rep", [P, QD], DT),
        ("xv_rep", [P, C], DT),
        ("mask", [NMASK, P, CHUNK], DT),
    ]
    if apply_gb:
        specs += [("gq_rep", [P, QD], DT), ("bq_rep", [P, QD], DT),
                  ("gk_rep", [P, QD], DT), ("bk_rep", [P, QD], DT),
                  ("gv_rep", [P, C], DT), ("bv_rep", [P, C], DT)]
    return specs


def get_nc(apply_gb, bf16=True, nrep=1, phases=4):
    key = (bool(apply_gb), bool(bf16), int(nrep), int(phases))
    if key in _NC_CACHE:
        return _NC_CACHE[key]
    import concourse.mybir as mybir
    import concourse.tile as tile
    from concourse import bacc

    nc = bacc.Bacc("TRN2", target_bir_lowering=False, debug=False,
                   num_devices=N_CORES)
    a = {}
    for name, shape, dt in _input_specs(apply_gb, bf16):
        a[name] = nc.dram_tensor(name, shape, dt, kind="ExternalInput").ap()
    a["out"] = nc.dram_tensor("out", [TQ, C], mybir.dt.float32,
                              kind="ExternalOutput").ap()
    with tile.TileContext(nc) as tc:
        _build_program(nc, tc, a, apply_gb, bf16, nrep=nrep, phases=phases)
    nc.compile()
    _NC_CACHE[key] = nc
    return nc


def _parity_mask(parity):
    m = np.zeros((NMASK, P, CHUNK), np.float32)
    for (s, kt), mi in MASK_IDX.items():
        qs = CHUNKS[parity][s] * CHUNK
        kg = np.arange(P, dtype=np.int64)[:, None] + P * kt
        qg = np.arange(CHUNK, dtype=np.int64)[None, :] + qs
        m[mi] = np.where(qg >= kg, 0.0, NEG).astype(np.float32)
    return m


def make_in_maps(inputs, bf16=True):
    import ml_dtypes
    cdt = ml_dtypes.bfloat16 if bf16 else np.float32

    x = np.asarray(inputs["x"], np.float32)
    idx = np.asarray(inputs["idx"]).astype(np.int64)
    k_tab = np.asarray(inputs["k_emb_tab"], np.float32)
    v_tab = np.asarray(inputs["v_emb_tab"], np.float32)
    W_qq = np.asarray(inputs["W_qq"], np.float32)
    W_k = np.asarray(inputs["W_k"], np.float32)
    W_kup = np.asarray(inputs["W_kup"], np.float32)
    W_v = np.asarray(inputs["W_v"], np.float32)
    W_vup = np.asarray(inputs["W_vup"], np.float32)
    x_q = np.asarray(inputs["x_q"], np.float32).reshape(QD)
    x_k = np.asarray(inputs["x_k"], np.float32).reshape(QD)
    x_v = np.asarray(inputs["x_v"], np.float32).reshape(C)
    g_q = np.asarray(inputs["g_q"], np.float32).reshape(QD)
    b_q = np.asarray(inputs["b_q"], np.float32).reshape(QD)
    g_k = np.asarray(inputs["g_k"], np.float32).reshape(QD)
    b_k = np.asarray(inputs["b_k"], np.float32).reshape(QD)
    g_v = np.asarray(inputs["g_v"], np.float32).reshape(C)
    b_v = np.asarray(inputs["b_v"], np.float32).reshape(C)

    apply_gb = not (np.all(g_q == 1) and np.all(b_q == 0)
                    and np.all(g_k == 1) and np.all(b_k == 0)
                    and np.all(g_v == 1) and np.all(b_v == 0))

    k_emb = k_tab[idx]          # [B, T, QD]
    v_emb = v_tab[idx]          # [B, T, C]
    vemb1 = [np.ascontiguousarray(v_emb[b] * (1.0 - x_v)).astype(cdt)
             for b in range(B)]
    vemb2 = [np.ascontiguousarray(v_emb[b] * x_v).astype(cdt)
             for b in range(B)]

    def cvt(arr):
        return np.ascontiguousarray(arr).astype(cdt)

    shared = {
        "wqq": cvt(W_qq.T),
        "wkv": cvt(np.concatenate([W_k, W_v], 0).T),
        "wkup": cvt(W_kup.T),
        "wvup": cvt(W_vup.T),
        "xq_rep": cvt(np.broadcast_to(x_q, (P, QD))),
        "xk_rep": cvt(np.broadcast_to(x_k, (P, QD))),
        "xv_rep": cvt(np.broadcast_to(x_v, (P, C))),
    }
    if apply_gb:
        for nm, v in [("gq", g_q), ("bq", b_q), ("gk", g_k), ("bk", b_k)]:
            shared[nm + "_rep"] = cvt(np.broadcast_to(v, (P, QD)))
        for nm, v in [("gv", g_v), ("bv", b_v)]:
            shared[nm + "_rep"] = cvt(np.broadcast_to(v, (P, C)))

    pmask = [_parity_mask(0).astype(cdt), _parity_mask(1).astype(cdt)]
    in_maps = []
    for c in range(N_CORES):
        b, parity = c // 2, c % 2
        chunks = CHUNKS[parity]
        cols = np.concatenate([np.arange(ch * CHUNK, (ch + 1) * CHUNK)
                               for ch in chunks])
        xqprev = np.zeros((NSLOT, C), np.float32)
        for j, ch in enumerate(chunks):
            if ch > 0:
                xqprev[j] = x[b, ch * CHUNK - 1]
        m = dict(shared)
        m.update(
            xT=cvt(x[b].T), xqT=cvt(x[b][cols].T),
            xqprevT=cvt(xqprev.T),
            kemb=cvt(k_emb[b]),
            vemb1=vemb1[b], vemb2=vemb2[b],
            mask=pmask[parity],
        )
        in_maps.append(m)
    return in_maps, apply_gb


def assemble_output(results):
    out = np.empty((B, T, C), np.float32)
    for c in range(N_CORES):
        oc = results[c]["out"]
        for j, ch in enumerate(CHUNKS[c % 2]):
            out[c // 2, ch * CHUNK:(ch + 1) * CHUNK] = \
                oc[j * CHUNK:(j + 1) * CHUNK]
    return out


BF16 = True


def kernel(**inputs):
    from concourse.bass_utils import run_bass_kernel_spmd
    in_maps, apply_gb = make_in_maps(inputs, bf16=BF16)
    nc = get_nc(apply_gb, bf16=BF16)
    res = run_bass_kernel_spmd(nc, in_maps, core_ids=list(range(N_CORES)))
    return assemble_output(res.results)
